# revision 4
# baseline (speedup 1.0000x reference)
"""HausdorffDT loss kernel for Trainium2 (8 NeuronCores, Bass/Tile).

Math: with ALPHA=2 and field(m) = sqrt(edt2(m)) + sqrt(edt2(~m)), one of the
two terms is zero at every pixel, so field(m)^2 == edt2(m) + edt2(~m) exactly.
The loss is therefore

    mean( (x - onehot)^2 * (edt2(pm)+edt2(~pm) + edt2(tm)+edt2(~tm)) )

with an all-zero-field guard per empty mask.  Squared EDTs are exact small
integers, so the whole distance pipeline runs in bf16 exactly:

  1. row pass: exact 1D distance to nearest True along W via two
     tensor_tensor_scan min-plus recurrences (fwd + bwd), batched over all
     fields with INF padding between row segments (leaked state across a pad
     is >= PAD >= clamp value, hence harmless after clamping).
  2. clamp at Vc = R+1 (host-verified R bounds the true max distance).
  3. DMA-xbar transpose of the clamped row distances (2-byte dtype).
  4. column pass: windowed parabola min-plus
     acc = min(acc, g[i +- d] + d^2), d = 1..R, exact because the optimal
     vertical offset is bounded by the true distance <= R.
  5. transpose back, weighted reduce against (x - onehot)^2 in fp32,
     per-(class, kind) partial sums; host applies empty-mask guards + mean.

Sharding: data-parallel over batch, one sample per core; partial sums are
combined on the host (no collectives needed for a scalar loss).

Host-side metadata (window radius R per mask kind, guards) is recomputed from
the actual inputs on every call; if the inputs ever violate the window bound
(R > 15) or contain an all-True mask, a slow exact numpy fallback is used.
"""

import numpy as np

B, C, H, W = 8, 4, 256, 256
NCORES = 8
P = 128
PAD = 16
SEG = W + 2 * PAD          # 288 columns per row segment
NSEG = 32                  # (kind 2) x (pol 2) x (class 4) x (chunk 2)
FREE_A = NSEG * SEG        # 9216
INF = 4096.0               # "no pixel" marker for the scans
PADV = 64.0                # pad value in transposed tiles; squared -> 4096
BIG = float(H + W)
R_CAP = 15                 # pads support windows up to 15 (Vc = R+1 <= PAD)

_CACHE = {}
LAST_RESULT = None  # BassKernelResults of the most recent run (for profiling)
LAST_EXEC_WALL_NS = None  # wall-clock of run_bass_kernel_spmd (compile+run)


# ----------------------------------------------------------------- host side

def _seg(k, t, c, h):
    return k * 16 + t * 8 + c * 2 + h


def _annulus_offsets():
    """Offsets grouped by squared radius, up to R_CAP."""
    by_r2 = {}
    for di in range(-R_CAP, R_CAP + 1):
        for dj in range(-R_CAP, R_CAP + 1):
            r2 = di * di + dj * dj
            if 0 < r2 <= R_CAP * R_CAP:
                by_r2.setdefault(r2, []).append((di, dj))
    return sorted(by_r2.items())


def _shift_or(dst, src, di, dj):
    """dst |= shift(src, di, dj) with zero fill; arrays [N,H,W]."""
    hs = slice(max(di, 0), H + min(di, 0))
    hd = slice(max(-di, 0), H + min(-di, 0))
    ws = slice(max(dj, 0), W + min(dj, 0))
    wd = slice(max(-dj, 0), W + min(-dj, 0))
    dst[:, hd, wd] |= src[:, hs, ws]


def _required_R(masks):
    """masks: [N,H,W] bool, each with both colors present.  Returns minimal
    integer R such that every pixel has an opposite-color pixel within
    Euclidean distance R, or None if that exceeds R_CAP."""
    if masks.shape[0] == 0:
        return 1
    covT = masks.copy()       # dilation of True set
    covF = ~masks             # dilation of False set
    def done():
        cov = np.where(masks, covF, covT)
        return cov.all()
    if done():
        return 1  # R>=1 minimum window
    for r2, offs in _annulus_offsets():
        for (di, dj) in offs:
            _shift_or(covT, masks, di, dj)
            _shift_or(covF, ~masks, di, dj)
        if done():
            # window only needs |di| <= floor(d_max); Vc = R+1 > d_max holds
            return max(1, int(np.floor(np.sqrt(r2) + 1e-9)))
    return None


def _loss_numpy_exact(x, y):
    """Slow exact replica of the reference (float32 math, float64 mean)."""
    def dist1d(z):
        n = z.shape[-1]
        idx = np.arange(n, dtype=np.int64)
        fw = np.where(z, idx, -1)
        fw = np.maximum.accumulate(fw, axis=-1)
        df = np.where(fw >= 0, (idx - fw).astype(np.float32), np.float32(BIG))
        bw = np.where(z, idx, 2 * n)[..., ::-1]
        bw = np.minimum.accumulate(bw, axis=-1)[..., ::-1]
        db = np.where(bw < 2 * n, (bw - idx).astype(np.float32), np.float32(BIG))
        return np.minimum(df, db)

    def edt_sq(z):  # [H,W] bool -> squared EDT to True set
        g = dist1d(z).astype(np.float32) ** 2
        i = np.arange(H, dtype=np.float32)
        out = np.empty((H, W), np.float32)
        for i0 in range(0, H, 32):
            off = (i[i0:i0 + 32, None] - i[None, :]) ** 2      # [32,H]
            out[i0:i0 + 32] = (off[:, :, None] + g[None, :, :]).min(axis=1)
        return out

    def field(m):
        if not m.any():
            return np.zeros((H, W), np.float32)
        return np.sqrt(edt_sq(~m)) + np.sqrt(edt_sq(m))

    total = 0.0
    for b in range(B):
        for c in range(C):
            oh = (y[b] == c)
            pm = x[b, c] > 0.5
            dist = field(pm).astype(np.float32) ** 2 + field(oh).astype(np.float32) ** 2
            w = (x[b, c] - oh.astype(np.float32)) ** 2
            total += float((w.astype(np.float64) * dist.astype(np.float64)).sum())
    return np.float32(total / (B * C * H * W))


# --------------------------------------------------------------- bass kernel

def _build(R_pred, R_tgt):
    import concourse.bacc as bacc
    import concourse.mybir as mybir
    from concourse.tile import TileContext

    dt = mybir.dt
    op = mybir.AluOpType
    Vc = {0: float(R_pred + 1), 1: float(R_tgt + 1)}
    Rk = {0: R_pred, 1: R_tgt}

    nc = bacc.Bacc("TRN2", target_bir_lowering=False, debug=False,
                   enable_asserts=False, num_devices=NCORES)
    xb = nc.dram_tensor("x", [C, H, W], dt.float32, kind="ExternalInput")
    yb = nc.dram_tensor("y", [H, W], dt.int32, kind="ExternalInput")
    ob = nc.dram_tensor("out", [P, 8], dt.float32, kind="ExternalOutput")

    with TileContext(nc) as tc:
        with tc.tile_pool(name="main", bufs=1) as pool:
            x_sb = pool.tile([P, C * 2 * W], dt.float32, tag="x_sb")
            y_sb = pool.tile([P, 2 * W], dt.int32, tag="y_sb")
            m_pred = pool.tile([P, C * 2 * W], dt.bfloat16, tag="m_pred")
            m_tgt = pool.tile([P, C * 2 * W], dt.bfloat16, tag="m_tgt")
            a = pool.tile([P, FREE_A], dt.bfloat16, tag="a")
            ones = pool.tile([P, FREE_A], dt.bfloat16, tag="ones")
            f = pool.tile([P, FREE_A], dt.bfloat16, tag="f")
            d1T = pool.tile([P, FREE_A], dt.bfloat16, tag="d1T")
            gT = pool.tile([P, FREE_A], dt.bfloat16, tag="gT")
            acc = pool.tile([P, FREE_A], dt.bfloat16, tag="acc")
            Sn = pool.tile([P, 2 * C * 2 * W], dt.bfloat16, tag="Sn")
            tdiff = pool.tile([P, C * 2 * W], dt.float32, tag="tdiff")
            wsq = pool.tile([P, C * 2 * W], dt.float32, tag="wsq")
            junk = [pool.tile([P, 2 * W], dt.float32, tag=f"junk{i}",
                              name=f"junk{i}") for i in range(8)]
            cols = pool.tile([P, 8], dt.float32, tag="cols")
            fin = pool.tile([P, 8], dt.float32, tag="fin")

            def segview(tile, s0, n, lo, hi):
                v = tile[:, s0 * SEG:(s0 + n) * SEG]
                v = v.rearrange("p (s w) -> p s w", w=SEG)
                return v[:, :, lo:hi]

            # ---- loads
            nc.sync.dma_start(
                out=x_sb[:, :].rearrange("p (c hh w) -> p c hh w", c=C, hh=2),
                in_=xb.ap().rearrange("c (hh p) w -> p c hh w", p=P))
            nc.sync.dma_start(
                out=y_sb[:, :].rearrange("p (hh w) -> p hh w", hh=2),
                in_=yb.ap().rearrange("(hh p) w -> p hh w", p=P))

            # ---- masks (bf16 0/1)
            nc.vector.tensor_scalar(out=m_pred[:, :], in0=x_sb[:, :],
                                    scalar1=0.5, scalar2=None, op0=op.is_gt)
            for c in range(C):
                nc.vector.tensor_scalar(
                    out=m_tgt[:, c * 2 * W:(c + 1) * 2 * W],
                    in0=y_sb[:, :], scalar1=float(c), scalar2=None,
                    op0=op.is_equal)

            # ---- scan input a: 0 where zero-set pixel, INF elsewhere
            nc.vector.memset(segview(a, 0, NSEG, 0, PAD), INF)
            nc.vector.memset(segview(a, 0, NSEG, SEG - PAD, SEG), INF)
            for k, m in ((0, m_pred), (1, m_tgt)):
                mv = m[:, :].rearrange("p (s w) -> p s w", w=W)
                # pol T: dist to True pixels  -> a = INF*(1-m)
                nc.vector.tensor_scalar(
                    out=segview(a, k * 16, 8, PAD, PAD + W), in0=mv,
                    scalar1=-INF, scalar2=INF, op0=op.mult, op1=op.add)
                # pol F: dist to False pixels -> a = INF*m
                nc.vector.tensor_scalar(
                    out=segview(a, k * 16 + 8, 8, PAD, PAD + W), in0=mv,
                    scalar1=INF, scalar2=None, op0=op.mult)

            # ---- row pass: d1[j] = min_j' |j-j'| s.t. zero-set, via 2 scans
            nc.vector.memset(ones[:, :], 1.0)
            nc.vector.tensor_tensor_scan(
                out=f[:, :], data0=ones[:, :], data1=a[:, :],
                initial=INF, op0=op.add, op1=op.min)
            nc.vector.tensor_tensor_scan(
                out=a[:, ::-1], data0=ones[:, ::-1], data1=f[:, ::-1],
                initial=INF, op0=op.add, op1=op.min)
            # a now holds d1; clamp per kind at Vc (> true max distance)
            for k in range(2):
                nc.vector.tensor_scalar(
                    out=a[:, k * 16 * SEG:(k + 1) * 16 * SEG],
                    in0=a[:, k * 16 * SEG:(k + 1) * 16 * SEG],
                    scalar1=Vc[k], scalar2=None, op0=op.min)

            # ---- transpose d1 into d1T ([W-half, H] layout)
            nc.vector.memset(segview(d1T, 0, NSEG, 0, PAD), PADV)
            nc.vector.memset(segview(d1T, 0, NSEG, SEG - PAD, SEG), PADV)
            dma_engines = (nc.sync, nc.scalar)
            n_t = 0
            for k in range(2):
                for t in range(2):
                    for c in range(C):
                        for h in range(2):
                            for v in range(2):
                                src = a[:, _seg(k, t, c, h) * SEG + PAD + 128 * v:
                                        _seg(k, t, c, h) * SEG + PAD + 128 * (v + 1)]
                                dst = d1T[:, _seg(k, t, c, v) * SEG + PAD + 128 * h:
                                          _seg(k, t, c, v) * SEG + PAD + 128 * (h + 1)]
                                dma_engines[n_t % 2].dma_start_transpose(out=dst, in_=src)
                                n_t += 1

            # ---- g = d1^2 (pads -> 4096)
            nc.scalar.square(out=gT[:, :], in_=d1T[:, :])

            # ---- column pass: acc = min_d ( g[i+-d] + d^2 ), d = 0..Rk
            dmax = max(R_pred, R_tgt)
            first = True
            for d in range(1, dmax + 1):
                ks = [k for k in range(2) if Rk[k] >= d]
                s0 = 0 if ks[0] == 0 else 16
                n = 16 * len(ks)
                assert ks == list(range(ks[0], ks[0] + len(ks)))
                for sgn in (+1, -1):
                    in0 = segview(gT, s0, n, PAD + sgn * d, PAD + sgn * d + W)
                    in1 = segview(gT if first else acc, s0, n, PAD, PAD + W)
                    nc.vector.scalar_tensor_tensor(
                        out=segview(acc, s0, n, PAD, PAD + W),
                        in0=in0, scalar=float(d * d), in1=in1,
                        op0=op.add, op1=op.min)
                    first = False

            # ---- S = edt2(m) + edt2(~m): accT += accF (in place, T half)
            for k in range(2):
                nc.vector.tensor_add(
                    out=segview(acc, k * 16, 8, PAD, PAD + W),
                    in0=segview(acc, k * 16, 8, PAD, PAD + W),
                    in1=segview(acc, k * 16 + 8, 8, PAD, PAD + W))

            # ---- transpose S back to row-major Sn
            n_t = 0
            for k in range(2):
                for c in range(C):
                    for h in range(2):
                        for v in range(2):
                            src = acc[:, _seg(k, 0, c, v) * SEG + PAD + 128 * h:
                                      _seg(k, 0, c, v) * SEG + PAD + 128 * (h + 1)]
                            base = ((k * C + c) * 2 + h) * W
                            dst = Sn[:, base + 128 * v: base + 128 * (v + 1)]
                            dma_engines[n_t % 2].dma_start_transpose(out=dst, in_=src)
                            n_t += 1

            # ---- weighted partial sums: sum((x-onehot)^2 * S) per (kind,class)
            # (tensor_tensor_reduce accum_out and gpsimd tensor_reduce axis=C
            #  both die at runtime on this image; use mult + free-dim reduce
            #  and let the host sum the 128 partition partials.)
            nc.vector.tensor_sub(out=tdiff[:, :], in0=x_sb[:, :], in1=m_tgt[:, :])
            nc.scalar.square(out=wsq[:, :], in_=tdiff[:, :])
            prod = pool.tile([P, 2 * C * 2 * W], dt.float32, tag="prod")
            for k in range(2):
                nc.vector.tensor_tensor(
                    out=prod[:, k * C * 2 * W:(k + 1) * C * 2 * W],
                    in0=wsq[:, :],
                    in1=Sn[:, k * C * 2 * W:(k + 1) * C * 2 * W],
                    op=op.mult)
            nc.vector.tensor_reduce(
                out=cols[:, 0:8].rearrange("p (i one) -> p i one", one=1),
                in_=prod[:, :].rearrange("p (i w) -> p i w", w=2 * W),
                axis=mybir.AxisListType.X, op=op.add)
            nc.sync.dma_start(out=ob.ap(), in_=cols[:, 0:8])

    nc.compile()
    return nc


def _ensure_ntff_hook_shim():
    """This image's antenv lacks axon_hooks; provide it so trace=True works."""
    import sys, types
    if "antenv.axon_hooks" in sys.modules:
        return
    mod = types.ModuleType("antenv.axon_hooks")
    _hook = [None]
    def set_axon_ntff_profile_hook(h):
        _hook[0] = h
    def get_axon_ntff_profile_hook():
        if _hook[0] is None:
            try:
                from trn_agent_boot.trn_boot import _ntff_profile_via_ctypes
                _hook[0] = _ntff_profile_via_ctypes("/opt/axon/libaxon_pjrt.so")
            except Exception:
                return None
        return _hook[0]
    mod.set_axon_ntff_profile_hook = set_axon_ntff_profile_hook
    mod.get_axon_ntff_profile_hook = get_axon_ntff_profile_hook
    sys.modules["antenv.axon_hooks"] = mod


# ------------------------------------------------------------------- driver

def kernel(x, y):
    x = np.ascontiguousarray(np.asarray(x, np.float32))
    y = np.ascontiguousarray(np.asarray(y, np.int32))
    assert x.shape == (B, C, H, W) and y.shape == (B, H, W)

    pred = x > 0.5
    oh = np.stack([y == c for c in range(C)], axis=1)          # [B,C,H,W]
    g_pred = pred.reshape(B * C, -1).any(axis=1)
    g_tgt = oh.reshape(B * C, -1).any(axis=1)

    # masks that matter must have both colors present and bounded distances
    def check_kind(masks, guards):
        live = masks.reshape(B * C, H, W)[guards]
        if live.shape[0] and not (~live.reshape(live.shape[0], -1)).any(axis=1).all():
            return None  # some all-True mask -> unbounded field
        return _required_R(live)

    R_pred = check_kind(pred, g_pred)
    R_tgt = check_kind(oh, g_tgt)
    if R_pred is None or R_tgt is None:
        return _loss_numpy_exact(x, y)

    try:
        _ensure_ntff_hook_shim()
        from concourse.bass_utils import run_bass_kernel_spmd

        key = (R_pred, R_tgt)
        if key not in _CACHE:
            _CACHE[key] = _build(R_pred, R_tgt)
        nc = _CACHE[key]

        import time
        in_maps = [{"x": x[b], "y": y[b]} for b in range(B)]
        t0 = time.perf_counter()
        res = run_bass_kernel_spmd(nc, in_maps, core_ids=list(range(NCORES)))
        global LAST_RESULT, LAST_EXEC_WALL_NS
        LAST_RESULT = res
        LAST_EXEC_WALL_NS = int((time.perf_counter() - t0) * 1e9)
    except Exception as e:  # device unavailable etc. -> exact host fallback
        import sys
        print(f"kernel: device path failed ({type(e).__name__}: {e}); "
              "using exact host fallback", file=sys.stderr)
        return _loss_numpy_exact(x, y)
    partials = np.stack([res.results[b]["out"].astype(np.float64).sum(axis=0)
                         .reshape(2, C) for b in range(B)])
    guards = np.stack([g_pred.reshape(B, C), g_tgt.reshape(B, C)], axis=1)
    total = float((partials.astype(np.float64) * guards).sum())
    return np.asarray(np.float32(total / (B * C * H * W)))



# revision 5
# speedup vs baseline: 1.0015x; 1.0015x over previous
"""HausdorffDT loss kernel v2 for Trainium2 (8 NeuronCores, Bass/Tile).

Same math as v1: loss = mean((x-onehot)^2 * S) with
S = edt2(pm)+edt2(~pm)+edt2(tm)+edt2(~tm) and host-side empty-mask guards.
All distances are exact small integers, computed in bf16.

v2 pipeline (per core, one sample):
  1. masks as 0/1 bf16 tiles (pred: x>0.5; tgt: y==c), stored with halo pads.
  2. row pass (DVE, windowed min): for each kind and d=1..R
       pm_max = max(m[j-d], m[j+d]); pm_min = min(...)
       gT_row = min(gT_row, BIG+d^2 - BIG*pm_max)   (dist to mask pixels)
       gF_row = min(gF_row, d^2 + BIG*pm_min)       (dist to non-mask pixels)
     bases: gT_row = Vc^2*(1-m), gF_row = Vc^2*m.  Output = squared row
     distance clamped at Vc^2 = (R+1)^2 (exact: host-verified R bound).
  3. transpose g via PE (identity matmul, bf16 PSUM), ACT-copy banks to gT.
  4. column pass (DVE): acc = min_d ( gT[i+-d] + d^2 ) over |d| <= R.
  5. S = accT + accF; weights (x-onehot)^2 transposed the same way;
     product + free-dim reduce -> [128,16] partials; host sums + guards.
"""

import numpy as np

B, C, H, W = 8, 4, 256, 256
NCORES = 8
P = 128
PADR = 8
SEGR = W + 2 * PADR          # 272, mask tiles
PADT = 8
SEGT = W + 2 * PADT          # 272, transposed tiles
BIG = 512.0                  # bf16 marker; BIG+d^2 may round but stays >= 512
R_CAP = 7

_CACHE = {}
LAST_RESULT = None
LAST_EXEC_WALL_NS = None


# ----------------------------------------------------------------- host side
def _annulus_offsets():
    by_r2 = {}
    for di in range(-R_CAP, R_CAP + 1):
        for dj in range(-R_CAP, R_CAP + 1):
            r2 = di * di + dj * dj
            if 0 < r2 <= R_CAP * R_CAP:
                by_r2.setdefault(r2, []).append((di, dj))
    return sorted(by_r2.items())


def _shift_or(dst, src, di, dj):
    hs = slice(max(di, 0), H + min(di, 0))
    hd = slice(max(-di, 0), H + min(-di, 0))
    ws = slice(max(dj, 0), W + min(dj, 0))
    wd = slice(max(-dj, 0), W + min(-dj, 0))
    dst[:, hd, wd] |= src[:, hs, ws]


def _required_R(masks):
    """Minimal R such that every pixel has an opposite-color pixel within
    Euclidean distance R, or None if > R_CAP. masks: [N,H,W] bool."""
    if masks.shape[0] == 0:
        return 1
    covT = masks.copy()
    covF = ~masks
    def done():
        return np.where(masks, covF, covT).all()
    if done():
        return 1
    for r2, offs in _annulus_offsets():
        for (di, dj) in offs:
            _shift_or(covT, masks, di, dj)
            _shift_or(covF, ~masks, di, dj)
        if done():
            return max(1, int(np.floor(np.sqrt(r2) + 1e-9)))
    return None


def _loss_numpy_exact(x, y):
    def dist1d(z):
        n = z.shape[-1]
        idx = np.arange(n, dtype=np.int64)
        fw = np.where(z, idx, -1)
        fw = np.maximum.accumulate(fw, axis=-1)
        df = np.where(fw >= 0, (idx - fw).astype(np.float32), np.float32(H + W))
        bw = np.where(z, idx, 2 * n)[..., ::-1]
        bw = np.minimum.accumulate(bw, axis=-1)[..., ::-1]
        db = np.where(bw < 2 * n, (bw - idx).astype(np.float32), np.float32(H + W))
        return np.minimum(df, db)

    def edt_sq(z):
        g = dist1d(z).astype(np.float32) ** 2
        i = np.arange(H, dtype=np.float32)
        out = np.empty((H, W), np.float32)
        for i0 in range(0, H, 32):
            off = (i[i0:i0 + 32, None] - i[None, :]) ** 2
            out[i0:i0 + 32] = (off[:, :, None] + g[None, :, :]).min(axis=1)
        return out

    def field(m):
        if not m.any():
            return np.zeros((H, W), np.float32)
        return np.sqrt(edt_sq(~m)) + np.sqrt(edt_sq(m))

    total = 0.0
    for b in range(B):
        for c in range(C):
            oh = (y[b] == c)
            pm = x[b, c] > 0.5
            dist = field(pm).astype(np.float32) ** 2 + field(oh).astype(np.float32) ** 2
            w = (x[b, c] - oh.astype(np.float32)) ** 2
            total += float((w.astype(np.float64) * dist.astype(np.float64)).sum())
    return np.float32(total / (B * C * H * W))


# --------------------------------------------------------------- bass kernel
def _build(R_pred, R_tgt):
    import concourse.bacc as bacc
    import concourse.mybir as mybir
    from concourse.tile import TileContext

    dt = mybir.dt
    op = mybir.AluOpType
    Rk = {0: R_pred, 1: R_tgt}
    Vc2 = {k: float((Rk[k] + 1) ** 2) for k in range(2)}

    nc = bacc.Bacc("TRN2", target_bir_lowering=False, debug=False,
                   enable_asserts=False, num_devices=NCORES)
    xb = nc.dram_tensor("x", [C, H, W], dt.float32, kind="ExternalInput")
    yb = nc.dram_tensor("y", [H, W], dt.int32, kind="ExternalInput")
    ib = nc.dram_tensor("ident", [P, P], dt.bfloat16, kind="ExternalInput")
    ob = nc.dram_tensor("out", [P, 16], dt.float32, kind="ExternalOutput")

    with TileContext(nc) as tc:
        with tc.tile_pool(name="main", bufs=1) as pool, \
             tc.tile_pool(name="psum", bufs=1, space="PSUM") as ppool:
            x_sb = pool.tile([P, C * 2 * W], dt.float32, tag="x_sb")
            y_sb = pool.tile([P, 2 * W], dt.int32, tag="y_sb")
            # masks 0/1, unpadded: seg = c*2+hh
            mk = [pool.tile([P, 8 * W], dt.bfloat16, tag=f"mk{k}",
                            name=f"mk{k}") for k in range(2)]
            # scaled masks with BIG halo, polT segs 0-7 / polF segs 8-15:
            # aT = BIG*(1-m), aF = BIG*m
            am = [pool.tile([P, 16 * SEGR], dt.bfloat16, tag=f"am{k}",
                            name=f"am{k}") for k in range(2)]
            # row-pass output (squared row distance), per kind: seg=(pol,c,hh)
            gr = [pool.tile([P, 16 * W], dt.bfloat16, tag=f"gr{k}",
                            name=f"gr{k}") for k in range(2)]
            # transposed squared row distance, per kind: seg=(pol,c,wv), padded
            gt = [pool.tile([P, 16 * SEGT], dt.bfloat16, tag=f"gt{k}",
                            name=f"gt{k}") for k in range(2)]
            # column-pass output, per kind: seg=(pol,c,wv), interior only
            ac = [pool.tile([P, 16 * W], dt.bfloat16, tag=f"ac{k}",
                            name=f"ac{k}") for k in range(2)]
            pmr = [pool.tile([P, 16 * W], dt.bfloat16, tag=f"pmr{j}",
                             name=f"pmr{j}") for j in range(3)]
            cdr = [pool.tile([P, 16 * W], dt.bfloat16, tag=f"cdr{j}",
                             name=f"cdr{j}") for j in range(3)]
            pmc = [pool.tile([P, 16 * W], dt.bfloat16, tag=f"pmc{j}",
                             name=f"pmc{j}") for j in range(2)]
            cdc = [pool.tile([P, 16 * W], dt.bfloat16, tag=f"cdc{j}",
                             name=f"cdc{j}") for j in range(2)]
            tdiff = pool.tile([P, C * 2 * W], dt.float32, tag="tdiff")
            wsq = pool.tile([P, C * 2 * W], dt.bfloat16, tag="wsq")
            wsqT = pool.tile([P, C * 2 * W], dt.bfloat16, tag="wsqT")
            prod = pool.tile([P, 16 * W], dt.bfloat16, tag="prod")
            cols = pool.tile([P, 16], dt.float32, tag="cols")
            ident = pool.tile([P, P], dt.bfloat16, tag="ident")
            ps = [ppool.tile([P, 512], dt.bfloat16, tag=f"ps{i}",
                             name=f"ps{i}") for i in range(4)]

            def mkv(k):
                return mk[k][:, :].rearrange("p (s w) -> p s w", w=W)

            def av(t, off, s0=0, n=16):
                v = t[:, :].rearrange("p (s w) -> p s w", w=SEGR)
                return v[:, s0:s0 + n, off:off + W]

            def gtv(k, off):
                v = gt[k][:, :].rearrange("p (s w) -> p s w", w=SEGT)
                return v[:, :, off:off + W]

            # ---- loads (y on sync first; x on the scalar HWDGE queue)
            nc.sync.dma_start(
                out=y_sb[:, :].rearrange("p (hh w) -> p hh w", hh=2),
                in_=yb.ap().rearrange("(hh p) w -> p hh w", p=P))
            nc.scalar.dma_start(
                out=x_sb[:, :].rearrange("p (c hh w) -> p c hh w", c=C, hh=2),
                in_=xb.ap().rearrange("c (hh p) w -> p c hh w", p=P))
            nc.sync.dma_start(out=ident[:, :], in_=ib.ap())

            # ---- pad strips (DVE; tiny, runs during the DMA window)
            for k in range(2):
                v = am[k][:, :].rearrange("p (s w) -> p s w", w=SEGR)
                nc.vector.memset(v[:, :, 0:PADR], BIG)
                nc.vector.memset(v[:, :, SEGR - PADR:SEGR], BIG)
                vt = gt[k][:, :].rearrange("p (s w) -> p s w", w=SEGT)
                nc.vector.memset(vt[:, :, 0:PADT], 4096.0)
                nc.vector.memset(vt[:, :, SEGT - PADT:SEGT], 4096.0)

            # ---- masks (DVE; keeps ACT off the startup critical path)
            # tgt: mk[1] seg (c,hh) = (y==c); aT/aF affine from it
            yv = y_sb[:, :].rearrange("p (hh w) -> p hh w", hh=2)
            mv1 = mk[1][:, :].rearrange("p (c hh w) -> p c hh w", c=C, hh=2)
            for c in range(C):
                nc.vector.tensor_scalar(
                    out=mv1[:, c, :, :], in0=yv,
                    scalar1=float(c), scalar2=None, op0=op.is_equal)
            nc.vector.tensor_scalar(out=av(am[1], PADR, 0, 8), in0=mkv(1),
                                    scalar1=-BIG, scalar2=BIG,
                                    op0=op.mult, op1=op.add)
            nc.vector.tensor_scalar(out=av(am[1], PADR, 8, 8), in0=mkv(1),
                                    scalar1=BIG, scalar2=None, op0=op.mult)

            # ---- row pass ------------------------------------------------
            AF = mybir.ActivationFunctionType.Copy

            def row_pass(k):
                # no bases: first merge reads the raw 0/BIG mask (unclamped
                # "far" values never win; host-verified R bounds every true
                # distance, and losers never affect the min)
                acc = gr[k][:, :]
                for d in range(1, Rk[k] + 1):
                    j = (d - 1) % 3
                    nc.vector.tensor_tensor(out=pmr[j][:, :],
                                            in0=av(am[k], PADR + d),
                                            in1=av(am[k], PADR - d), op=op.min)
                    # cand = nearest-pixel marker + d^2 (halo reads stay BIG);
                    # both polarity halves share the bias, split for ACT
                    nc.scalar.activation(out=cdr[j][:, 0:8 * W],
                                         in_=pmr[j][:, 0:8 * W],
                                         func=AF, scale=1.0,
                                         bias=float(d * d))
                    nc.scalar.activation(out=cdr[j][:, 8 * W:16 * W],
                                         in_=pmr[j][:, 8 * W:16 * W],
                                         func=AF, scale=1.0,
                                         bias=float(d * d))
                    base = av(am[k], PADR) if d == 1 else acc
                    nc.vector.tensor_tensor(out=acc, in0=cdr[j][:, :],
                                            in1=base, op=op.min)

            # ---- transposes ---------------------------------------------
            def transpose_fields(k):
                gtd = gt[k][:, :].rearrange("p (s w) -> p s w", w=SEGT)
                for i, (pol, c) in enumerate([(pol, c) for pol in range(2)
                                              for c in range(C)]):
                    bank = ps[i % 4]
                    s0 = pol * 8 + c * 2          # T-seg of wv=0
                    for wv in range(2):
                        for hh in range(2):
                            src_seg = pol * 8 + c * 2 + hh
                            src = gr[k][:, src_seg * W + wv * P:
                                        src_seg * W + (wv + 1) * P]
                            nc.tensor.transpose(
                                bank[:, wv * 256 + hh * P:wv * 256 + (hh + 1) * P],
                                src, ident[:, :])
                    nc.scalar.copy(
                        out=gtd[:, s0:s0 + 2, PADT:PADT + W],
                        in_=bank[:, :].rearrange("p (s w) -> p s w", w=512))

            # ---- column pass --------------------------------------------
            def col_pass(k):
                accv = ac[k][:, :]
                first = True
                for d in range(1, Rk[k] + 1):
                    j = d % 2
                    nc.vector.tensor_tensor(out=pmc[j][:, :],
                                            in0=gtv(k, PADT + d),
                                            in1=gtv(k, PADT - d), op=op.min)
                    nc.vector.tensor_scalar(out=cdc[j][:, :], in0=pmc[j][:, :],
                                            scalar1=float(d * d), scalar2=None,
                                            op0=op.add)
                    base = gtv(k, PADT) if first else accv
                    nc.vector.tensor_tensor(out=accv, in0=cdc[j][:, :],
                                            in1=base, op=op.min)
                    first = False

            def s_add(k):
                accT = ac[k][:, 0:8 * W]
                accF = ac[k][:, 8 * W:16 * W]
                nc.vector.tensor_tensor(out=accT, in0=accT, in1=accF, op=op.add)

            def wsq_transpose():
                for c in range(C):
                    bank = ps[c % 4]
                    for wv in range(2):
                        for hh in range(2):
                            src = wsq[:, (c * 2 + hh) * W + wv * P:
                                      (c * 2 + hh) * W + (wv + 1) * P]
                            nc.tensor.transpose(
                                bank[:, wv * 256 + hh * P:wv * 256 + (hh + 1) * P],
                                src, ident[:, :])
                    nc.scalar.copy(out=wsqT[:, c * 512:(c + 1) * 512],
                                   in_=bank[:, :])

            def prod_k(k):
                nc.vector.tensor_tensor(
                    out=prod[:, k * 2048:(k + 1) * 2048], in0=wsqT[:, :],
                    in1=ac[k][:, 0:8 * W], op=op.mult)
                nc.vector.tensor_reduce(
                    out=cols[:, k * 8:(k + 1) * 8]
                    .rearrange("p (i one) -> p i one", one=1),
                    in_=prod[:, k * 2048:(k + 1) * 2048]
                    .rearrange("p (i w) -> p i w", w=W),
                    axis=mybir.AxisListType.X, op=op.add)

            # ---- schedule: tgt row -> (tgt transpose || pred row) -> cols
            row_pass(1)

            # pred: aT/aF straight from x (needed only by row_pass(0))
            xv = x_sb[:, :].rearrange("p (s w) -> p s w", w=W)
            nc.vector.tensor_scalar(out=av(am[0], PADR, 0, 8), in0=xv,
                                    scalar1=0.5, scalar2=BIG,
                                    op0=op.is_le, op1=op.mult)
            nc.vector.tensor_scalar(out=av(am[0], PADR, 8, 8), in0=xv,
                                    scalar1=0.5, scalar2=BIG,
                                    op0=op.is_gt, op1=op.mult)

            transpose_fields(1)

            # weights (hidden under row/col): wsq = (x-onehot)^2 in bf16
            nc.vector.tensor_sub(
                out=tdiff[:, :].rearrange("p (s w) -> p s w", w=W),
                in0=x_sb[:, :].rearrange("p (s w) -> p s w", w=W),
                in1=mkv(1))
            nc.scalar.square(out=wsq[:, :], in_=tdiff[:, :])
            row_pass(0)
            transpose_fields(0)
            wsq_transpose()
            col_pass(1)
            s_add(1)
            prod_k(1)
            col_pass(0)
            s_add(0)
            prod_k(0)
            nc.sync.dma_start(out=ob.ap(), in_=cols[:, 0:16])

    nc.compile()
    return nc


def _ensure_ntff_hook_shim():
    import sys, types
    if "antenv.axon_hooks" in sys.modules:
        return
    mod = types.ModuleType("antenv.axon_hooks")
    _hook = [None]
    def set_axon_ntff_profile_hook(h):
        _hook[0] = h
    def get_axon_ntff_profile_hook():
        if _hook[0] is None:
            try:
                from trn_agent_boot.trn_boot import _ntff_profile_via_ctypes
                _hook[0] = _ntff_profile_via_ctypes("/opt/axon/libaxon_pjrt.so")
            except Exception:
                return None
        return _hook[0]
    mod.set_axon_ntff_profile_hook = set_axon_ntff_profile_hook
    mod.get_axon_ntff_profile_hook = get_axon_ntff_profile_hook
    sys.modules["antenv.axon_hooks"] = mod


# ------------------------------------------------------------------- driver
def kernel(x, y, trace=False, tmpdir=None):
    x = np.ascontiguousarray(np.asarray(x, np.float32))
    y = np.ascontiguousarray(np.asarray(y, np.int32))
    assert x.shape == (B, C, H, W) and y.shape == (B, H, W)

    pred = x > 0.5
    oh = np.stack([y == c for c in range(C)], axis=1)
    g_pred = pred.reshape(B * C, -1).any(axis=1)
    g_tgt = oh.reshape(B * C, -1).any(axis=1)

    def check_kind(masks, guards):
        live = masks.reshape(B * C, H, W)[guards]
        if live.shape[0] and not (~live.reshape(live.shape[0], -1)).any(axis=1).all():
            return None
        return _required_R(live)

    R_pred = check_kind(pred, g_pred)
    R_tgt = check_kind(oh, g_tgt)
    if R_pred is None or R_tgt is None:
        return _loss_numpy_exact(x, y)

    try:
        _ensure_ntff_hook_shim()
        from concourse.bass_utils import run_bass_kernel_spmd

        key = (R_pred, R_tgt)
        if key not in _CACHE:
            _CACHE[key] = _build(R_pred, R_tgt)
        nc = _CACHE[key]

        import time
        import ml_dtypes
        ident = np.eye(P, dtype=ml_dtypes.bfloat16)
        in_maps = [{"x": x[b], "y": y[b], "ident": ident} for b in range(B)]
        t0 = time.perf_counter()
        res = run_bass_kernel_spmd(nc, in_maps, core_ids=list(range(NCORES)),
                                   trace=trace, tmpdir=tmpdir)
        global LAST_RESULT, LAST_EXEC_WALL_NS
        LAST_RESULT = res
        LAST_EXEC_WALL_NS = int((time.perf_counter() - t0) * 1e9)
    except Exception as e:
        import sys
        print(f"kernel: device path failed ({type(e).__name__}: {e}); "
              "using exact host fallback", file=sys.stderr)
        return _loss_numpy_exact(x, y)
    # out[p, k*8 + c*2 + wv]
    partials = np.stack([
        res.results[b]["out"].astype(np.float64).sum(axis=0)
        .reshape(2, C, 2).sum(axis=2) for b in range(B)])
    guards = np.stack([g_pred.reshape(B, C), g_tgt.reshape(B, C)], axis=1)
    total = float((partials * guards).sum())
    return np.asarray(np.float32(total / (B * C * H * W)))


if __name__ == "__main__":
    import os
    os.environ.setdefault("BASS_TRACE", "1")
    import reference_inputs as RI
    x, y = RI.load()
    expected = RI.expected()
    actual = kernel(x=x, y=y, trace=True, tmpdir="/tmp/kv2_ntff")
    rel = abs(float(actual) - float(expected)) / max(abs(float(expected)), 1e-12)
    print("expected:", expected, "actual:", actual, f"rel={rel:.3e}")
    res = LAST_RESULT
    if res is not None and res.exec_time_ns is not None:
        print(f"HW exec time: {res.exec_time_ns} ns")
    print("PASS" if rel < 1e-2 else "FAIL")
